# revision 19
# baseline (speedup 1.0000x reference)
"""BitLinear (binary group-scaled quantized linear) TRN2 Bass kernel.

y = x @ (sign(w) * s).T + bias, s = max(|scale_group|, 1e-8) per 128-elem
group of flattened w.  Shapes: x [4,2048,4096], w [11008,4096],
bias [11008], scale [352256] -> y [4,2048,11008].

Sharding: column-parallel over out_features across 8 cores (1376 each).
x is replicated (host pre-transposed), w/scale/bias sliced. No collectives.

Precision: hybrid. k-tiles 0..23 run fp16 x * fp16 w_bin (1 PE row/cycle).
k-tiles 24..31 run fp8 e4m3 x * fp8 w_bin via DoubleRow matmuls (2 rows/
cycle), cutting PE cycles 12.5%. Predicted L2 rel err ~1.7e-2 (< 2e-2 gate):
fp8 rounding of x (~2.6% rms) and of the group scale (~2.6% rms) over 1/4
of the contraction.
"""

import os
import sys

for _p in ("/opt/trn_rl_repo",):
    if _p not in sys.path and os.path.isdir(_p):
        sys.path.insert(0, _p)

import numpy as np

import concourse.bass as bass
import concourse.mybir as mybir
import concourse.tile as tile
from concourse import bacc
from concourse.bass_utils import run_bass_kernel_spmd

P = 128
N_CORES = 8

# Problem shape (hardcoded per spec nn_BitLinear_65506841199020)
B, S, IN, OUT = 4, 2048, 4096, 11008
T = B * S                      # 8192 rows of x
O_SH = OUT // N_CORES          # 1376 out features per core
K = IN                         # 4096 contraction
KT = K // P                    # 32 k-tiles
KT8 = 8                        # trailing k-tiles in fp8 DoubleRow
KT16 = KT - KT8                # leading k-tiles in fp16
GROUP = 128                    # quant group size == P
EPS = 1e-8

TCH = 256                      # t-columns per x strip chunk
F16 = mybir.dt.float16
BF16 = mybir.dt.bfloat16
F32 = mybir.dt.float32
FP8 = mybir.dt.float8e4

LAST_EXEC_NS = None
_NC_CACHE = {}


def _o_blocks(o_sh, blk=512):
    out, o = [], 0
    while o < o_sh:
        w = min(blk, o_sh - o)
        out.append((o, w))
        o += w
    return out


def _emit(nc, tc, xT, xT8, wT, scaleT, bias_t, y, t_dim, o_sh, tch):
    """Tile kernel body. xT [KT16*P, t_dim] f16, xT8 [KT8*P, t_dim] fp8e4,
    wT [KT*P, o_sh] bf16, scaleT [KT, o_sh] f16 (rows >= KT16 pre-rounded
    to the e4m3 grid), bias [o_sh] f32, y [t_dim, o_sh] f32."""
    import contextlib

    o_blocks = _o_blocks(o_sh)

    with contextlib.ExitStack() as ctx:
        const = ctx.enter_context(tc.tile_pool(name="const", bufs=1))
        wload = ctx.enter_context(tc.tile_pool(name="wload", bufs=6))
        sgnp = ctx.enter_context(tc.tile_pool(name="sgn", bufs=4))
        sbc = ctx.enter_context(tc.tile_pool(name="sbc", bufs=6))
        rowp = ctx.enter_context(tc.tile_pool(name="rows", bufs=8))
        wbinp = ctx.enter_context(tc.tile_pool(name="wbin", bufs=1))
        xsp = ctx.enter_context(tc.tile_pool(name="xs", bufs=3))
        stage = ctx.enter_context(tc.tile_pool(name="stage", bufs=6))
        psum = ctx.enter_context(tc.tile_pool(name="psum", bufs=8, space="PSUM"))

        def load_strip(tci, splits=None):
            # issued from GpSimd (otherwise idle): keeps the sync engine's
            # DMA queue short — each dma_start costs ~0.65us issue time on
            # its engine, and w/y DMAs stay latency-critical on sync.
            # xT/xT8 are chunk-major [P, n_ch, kt, tch] so one strip is a
            # single contiguous block per partition (large DMA packets).
            xs = xsp.tile([P, KT16, tch], F16, name=f"xs{tci % 3}", tag="xs")
            x8 = xsp.tile([P, KT8, tch], FP8, name=f"x8{tci % 3}", tag="x8")
            for d, ke in (splits or [(0, KT16)]):
                nc.gpsimd.dma_start(
                    out=xs[:, d:ke, :], in_=xT[:, tci, d:ke, :]
                )
            nc.gpsimd.dma_start(out=x8[:], in_=xT8[:, tci, :, :])
            return xs, x8

        n_ch = t_dim // tch
        n_sub = tch // P
        nblk = len(o_blocks)
        n_rounds = n_ch * n_sub

        # strip 0 queued before the quantize DMAs so the first matmuls can
        # start as soon as wbin[0] lands (queues are FIFO per engine);
        # a small first slice = lower latency for the k=0 subtile the
        # first MM needs
        strips = {0: load_strip(0, splits=[(0, 2), (2, 8), (8, 16), (16, KT16)])}

        # scale rows arrive as tiny [1, o_sh] DMAs and are fanned out to all
        # 128 partitions by GpSimd partition_broadcast — keeping the fat
        # 352KB broadcast writes off the DMA channels entirely. The first
        # few rows are hoisted ahead of the strip-1 load so their
        # broadcasts clear GpSimd before its queue fills with strip DMAs.
        def scale_row(ki, eng):
            r = rowp.tile([1, o_sh], F16, name=f"r{ki % 8}", tag="r")
            eng.dma_start(out=r[:], in_=scaleT[ki:ki + 1, :])
            return r

        def scale_bcast(ki, row):
            sb = sbc.tile([P, o_sh], F16, name="sb", tag="sb")
            nc.gpsimd.partition_broadcast(sb[:], row[:])
            return sb

        N_HOIST = 4
        sb_pre = {}
        for ki in range(N_HOIST):
            sb_pre[ki] = scale_bcast(ki, scale_row(ki, nc.scalar))

        # bias broadcast to all partitions: [P, o_sh], same row+broadcast
        # trick (the 704KB DMA version hogged a channel for ~15us)
        bias_sb = const.tile([P, o_sh], F32)
        bias_row = rowp.tile([1, o_sh], F32, name="brow", tag="br", bufs=1)
        nc.scalar.dma_start(out=bias_row[:], in_=bias_t[:])

        # fp8 binary weights for k-tiles KT16..KT-1, pair-sliceable for
        # DoubleRow: [P, KT8, o_sh]
        wb8 = const.tile([P, KT8, o_sh], FP8)

        def evict_blocks(ps, trow, blocks, engs=None):
            for bi, (o0, ow) in enumerate(blocks):
                st = stage.tile([P, 512], F32, name=f"st{bi}", tag="st")
                nc.vector.tensor_tensor(
                    out=st[:, :ow], in0=ps[bi][:, :ow],
                    in1=bias_sb[:, o0:o0 + ow], op=mybir.AluOpType.add,
                )
                eng = engs[bi % len(engs)] if engs else nc.sync
                eng.dma_start(
                    out=y[trow:trow + P, o0:o0 + ow], in_=st[:, :ow]
                )

        def lhsT_of(s):
            ch, sub = divmod(s, n_sub)
            return strips[ch], sub

        def chunk_mms(ps, x8_s, sub, blocks, c):
            # one DoubleRow fp8 matmul pair (k-tiles KT16+2c, KT16+2c+1)
            lhsT = x8_s[:, 2 * c:2 * c + 2, sub * P:(sub + 1) * P]
            for bi, (o0, ow) in enumerate(blocks):
                nc.tensor.matmul(
                    ps[bi][:, :ow], lhsT, wb8[:, 2 * c:2 * c + 2, o0:o0 + ow],
                    start=False, stop=(c == KT8 // 2 - 1),
                    perf_mode=mybir.MatmulPerfMode.DoubleRow,
                )

        # During quantize, PSUM banks cap how much matmul work can overlap.
        # Run NARROW rounds (first 2 o-blocks = 2 banks) for the first 4
        # t-subtiles — 8 banks exactly — so PE consumption (~1.7us/ktile)
        # tracks wbin arrival; the left-over o-block runs densely right
        # after as 1-bank full-k rounds.
        a_blocks = o_blocks[:2] if nblk >= 2 else o_blocks
        b_blocks = o_blocks[len(a_blocks):]
        a_subs = min(4 if nblk >= 2 else 2, n_rounds, 8 // len(a_blocks))
        for c in range(1, (a_subs + n_sub - 1) // n_sub):
            strips[c] = load_strip(c, splits=[(0, 4), (4, KT16)])
        fused = [
            [
                psum.tile([P, 512], F32, name=f"fps{s}_{bi}", tag="ps")
                for bi in range(len(a_blocks))
            ]
            for s in range(a_subs)
        ]

        # ---- quantize: w_binT[ki] = sign(w) * max(scale, eps) ----
        # ki < KT16: fp16 (scale arrives pre-cast fp16 > 0;
        # fp16(sign*s_f32) == sign*fp16(s)). ki >= KT16: fp8 e4m3 (scale
        # rows pre-rounded to the e4m3 grid host-side, so sign*s is an
        # exact fp8 value and the DVE fp8 writeback is exact).
        wbin = []
        for ki in range(KT):
            wt = wload.tile([P, o_sh], BF16, name="wt", tag="wt")
            # single full-width DMA: splitting shrinks the per-partition
            # packet (2752B -> 688B) and is ~4x slower end-to-end.
            # Alternate HWDGE channels to halve per-channel serialization.
            w_eng = nc.sync if ki % 2 == 0 else nc.scalar
            w_eng.dma_start(out=wt[:], in_=wT[ki * P:(ki + 1) * P, :])
            if ki == 3:
                nc.gpsimd.partition_broadcast(bias_sb[:], bias_row[:])
            # scale arrives host-side pre-maxed (max(|s|, eps)) so no DVE
            # max is needed
            if ki in sb_pre:
                sb = sb_pre[ki]
            else:
                sb = scale_bcast(ki, scale_row(ki, nc.sync))
            sg = sgnp.tile([P, o_sh], F16, name="sg", tag="sg")
            nc.scalar.activation(
                out=sg[:], in_=wt[:], func=mybir.ActivationFunctionType.Sign
            )
            if ki < KT16:
                wb = wbinp.tile([P, o_sh], F16, name=f"wb{ki}", tag=f"wbin{ki}")
                nc.vector.tensor_mul(out=wb[:], in0=sg[:], in1=sb[:])
                wbin.append(wb)
                for s in range(a_subs):
                    (xs_s, _), sub = lhsT_of(s)
                    lhsT = xs_s[:, ki, sub * P:(sub + 1) * P]
                    for bi, (o0, ow) in enumerate(a_blocks):
                        nc.tensor.matmul(
                            fused[s][bi][:, :ow], lhsT, wb[:, o0:o0 + ow],
                            start=(ki == 0), stop=False,
                        )
            else:
                nc.vector.tensor_mul(
                    out=wb8[:, ki - KT16, :], in0=sg[:], in1=sb[:]
                )
                if (ki - KT16) % 2 == 1:
                    c = (ki - KT16) // 2
                    for s in range(a_subs):
                        (_, x8_s), sub = lhsT_of(s)
                        chunk_mms(fused[s], x8_s, sub, a_blocks, c)
        for s in range(a_subs):
            _, sub = lhsT_of(s)
            evict_blocks(fused[s], (s // n_sub) * tch + sub * P, a_blocks)

        def full_k(ps, xs_s, x8_s, sub, blocks):
            for ki in range(KT16):
                lhsT = xs_s[:, ki, sub * P:(sub + 1) * P]
                for bi, (o0, ow) in enumerate(blocks):
                    nc.tensor.matmul(
                        ps[bi][:, :ow], lhsT, wbin[ki][:, o0:o0 + ow],
                        start=(ki == 0), stop=False,
                    )
            for c in range(KT8 // 2):
                chunk_mms(ps, x8_s, sub, blocks, c)

        # left-over o-range of the startup subtiles: dense full-k rounds
        if b_blocks:
            nch_startup = (a_subs + n_sub - 1) // n_sub
            if nch_startup < n_ch and nch_startup not in strips:
                strips[nch_startup] = load_strip(nch_startup)
            for s in range(a_subs):
                (xs_s, x8_s), sub = lhsT_of(s)
                ps = [
                    psum.tile([P, 512], F32, name=f"bp{bi}", tag="ps")
                    for bi in range(len(b_blocks))
                ]
                full_k(ps, xs_s, x8_s, sub, b_blocks)
                evict_blocks(ps, (s // n_sub) * tch + sub * P, b_blocks)

        # ---- remaining rounds: full o-width, 3 banks each ----
        for s in range(a_subs, n_rounds):
            ch, sub = divmod(s, n_sub)
            if ch not in strips:
                strips[ch] = load_strip(ch)
            # prefetch the next strip one chunk ahead so its DMA latency
            # hides behind this chunk's ~2 rounds of matmuls
            if sub == 0 and ch + 1 < n_ch and ch + 1 not in strips:
                strips[ch + 1] = load_strip(ch + 1)
            xs_s, x8_s = strips[ch]
            ps = [
                psum.tile([P, 512], F32, name=f"ps{bi}", tag="ps")
                for bi in range(nblk)
            ]
            full_k(ps, xs_s, x8_s, sub, o_blocks)
            evict_blocks(
                ps, ch * tch + sub * P, o_blocks, engs=(nc.sync, nc.scalar)
            )


def build_nc(t_dim=T, o_sh=O_SH, tch=TCH, debug=False):
    key = (t_dim, o_sh, tch, debug)
    if key in _NC_CACHE:
        return _NC_CACHE[key]
    nc = bacc.Bacc(
        "TRN2", target_bir_lowering=False, debug=debug, num_devices=N_CORES
    )
    n_ch = t_dim // tch
    xT = nc.dram_tensor("xT", [P, n_ch, KT16, tch], F16, kind="ExternalInput")
    xT8 = nc.dram_tensor("xT8", [P, n_ch, KT8, tch], FP8, kind="ExternalInput")
    wT = nc.dram_tensor("wT", [KT * P, o_sh], BF16, kind="ExternalInput")
    scaleT = nc.dram_tensor("scaleT", [KT, o_sh], F16, kind="ExternalInput")
    bias_t = nc.dram_tensor("bias", [1, o_sh], F32, kind="ExternalInput")
    y = nc.dram_tensor("y", [t_dim, o_sh], F32, kind="ExternalOutput")
    with tile.TileContext(nc) as tc:
        _emit(nc, tc, xT, xT8, wT, scaleT, bias_t, y, t_dim, o_sh, tch)
    nc.compile()
    _NC_CACHE[key] = nc
    return nc


def _prep_inputs(x, weight, bias, scale):
    """Host-side sharding/layout prep (dtype/layout only; the e4m3 grid
    rounding of x's fp8 k-range and of scale rows >= KT16 fixes the
    quantization grid the device kernel computes in)."""
    import ml_dtypes

    KS = KT16 * P
    NCH = T // TCH
    xTf = np.ascontiguousarray(x.reshape(T, K).T, dtype=np.float32)  # [K, T]
    # chunk-major layout [P, n_ch, kt, tch]: one t-strip is a contiguous
    # block per partition -> large DMA packets
    xT = np.ascontiguousarray(
        xTf[:KS].astype(np.float16)
        .reshape(KT16, P, NCH, TCH).transpose(1, 2, 0, 3)
    )
    xT8 = np.ascontiguousarray(
        xTf[KS:].astype(ml_dtypes.float8_e4m3)
        .reshape(KT8, P, NCH, TCH).transpose(1, 2, 0, 3)
    )
    # scale groups: group g of flattened w -> row o = g // (IN//GROUP),
    # k-tile ki = g % (IN//GROUP) since IN % GROUP == 0
    sc = scale[: OUT * (IN // GROUP)].reshape(OUT, IN // GROUP)
    sc = np.maximum(np.abs(sc), EPS)
    in_maps = []
    for c in range(N_CORES):
        o0 = c * O_SH
        wTc = np.ascontiguousarray(
            weight[o0:o0 + O_SH, :].T, dtype=np.float32
        )  # [K, O_SH]
        # bf16 cast preserves sign exactly (full fp32 exponent range)
        wTb = wTc.astype(ml_dtypes.bfloat16)
        scT = np.ascontiguousarray(
            sc[o0:o0 + O_SH, :].T, dtype=np.float32
        )  # [KT, O_SH]
        # fp8 k-tiles: pre-round the scale to the e4m3 grid (exact in fp16)
        scT[KT16:] = scT[KT16:].astype(ml_dtypes.float8_e4m3).astype(np.float32)
        in_maps.append({
            "xT": xT,
            "xT8": xT8,
            "wT": wTb,
            "scaleT": scT.astype(np.float16),
            "bias": np.ascontiguousarray(
                bias[o0:o0 + O_SH], dtype=np.float32
            ).reshape(1, O_SH),
        })
    return in_maps


def _install_ntff_hook_shim():
    """The agent image's antenv lacks axon_hooks (a get/set registry), so
    run_bass_kernel_spmd(trace=True) can't find the NTFF profile hook that
    trn_agent_boot would register. Recreate the registry + registration."""
    import types
    import antenv

    if "antenv.axon_hooks" in sys.modules:
        return
    mod = types.ModuleType("antenv.axon_hooks")
    mod._HOOK = None

    def set_axon_ntff_profile_hook(h):
        mod._HOOK = h

    def get_axon_ntff_profile_hook():
        return mod._HOOK

    mod.set_axon_ntff_profile_hook = set_axon_ntff_profile_hook
    mod.get_axon_ntff_profile_hook = get_axon_ntff_profile_hook
    sys.modules["antenv.axon_hooks"] = mod
    antenv.axon_hooks = mod
    try:
        if "/root/.axon_site" not in sys.path and os.path.isdir("/root/.axon_site"):
            sys.path.append("/root/.axon_site")
        from trn_agent_boot.trn_boot import _ntff_profile_via_ctypes

        hook = _ntff_profile_via_ctypes("/opt/axon/libaxon_pjrt.so")
        if hook is not None:
            set_axon_ntff_profile_hook(hook)
    except Exception as e:
        sys.stderr.write(f"ntff hook shim failed: {e!r}\n")


def kernel(x, weight, bias, scale):
    global LAST_EXEC_NS
    nc = build_nc()
    in_maps = _prep_inputs(
        np.asarray(x, dtype=np.float32),
        np.asarray(weight, dtype=np.float32),
        np.asarray(bias, dtype=np.float32),
        np.asarray(scale, dtype=np.float32),
    )
    core_ids = list(range(N_CORES))
    want_trace = os.environ.get("BITLIN_TRACE", "0") != "0"
    res = None
    if want_trace:
        try:
            _install_ntff_hook_shim()
            res = run_bass_kernel_spmd(nc, in_maps, core_ids, trace=True)
            LAST_EXEC_NS = res.exec_time_ns
        except Exception as e:  # fall back to untraced run
            sys.stderr.write(f"kernel: traced run failed ({e!r}); retrying\n")
            res = None
    if res is None:
        res = run_bass_kernel_spmd(nc, in_maps, core_ids)
        LAST_EXEC_NS = res.exec_time_ns
    y = np.concatenate(
        [res.results[c]["y"] for c in range(N_CORES)], axis=1
    )
    return np.ascontiguousarray(y.reshape(B, S, OUT), dtype=np.float32)
